# revision 14
# baseline (speedup 1.0000x reference)
"""Exact self-kNN (k=32) on 8 TRN2 NeuronCores — 2-pass PE, cast-quantized
packed scores, 1024-wide selection windows.

Structure per 1024-column window (16 windows x 16 query tiles per core):
  PE    : psum = 2^17 * x_i . x_j   (four fp16 matmuls into a 2-bank
          [128,1024] PSUM tile; q*2^9, db*2^8)
  ScalarE: s = int32(psum * 2^-12 + 2^5*A_i)  (ONE 1024-wide activation:
          the int32 output cast IS the score quantizer; A_i =
          (450-sq_i)/2 - 128 is per-row centering, ranking-neutral)
  X2    : p = float(s) + R_j  (tensor_tensor add; R_j =
          round(2^5*(128 - sq_j/2)) + j_local*2^-11 packs the column bias
          in integer units and the 10-bit local index in the fraction).
          Per window either VectorE (one 1024-wide mixed i32+f32 TT) or
          Pool (ACT recast to f32 + two 512-wide TTs; Pool rejects mixed
          dtypes and >512-wide ops are pathological on it)
  VectorE: max8 over the 1024-wide window -> 8 packed candidates
So p = 2^5*((450 - d_ij)/2)_q + j_local*2^-11, |p| < 2^13, fp32-exact.
A 5-round max8/max_index/match_replace merge over the 128 candidates
gives top-40; global index = (pos>>3)*1024 + (int32(p*2048) & 1023).
Slot 0 is always the self match and is overwritten with the row id.

Why this shape: the PE throttles to ~1.2 GHz effective only when its duty
cycle exceeds ~50%; with just the two irreducible dot passes it stays
under that and streams at 2.4 GHz.  Measured 512-wide costs: ACT
psum->sbuf ~710 ns (1024-wide ~1137), DVE max8 ~593/512 (~1187/1024),
DVE TT ~602 (1224/1024), Pool TT ~1232-1639.  The window widening cuts
the DVE max8+merge floor; the 2-bank cast cuts ACT; the 9/16 pool share
keeps Pool below its concurrency ceiling.

Host: exact fp32 distances for the 40 candidates per query, stable
(d, idx) sort, keep 32.
"""

import numpy as np

N = 16384
D = 256
K = 32
KDEV = 40                     # device returns top-40 candidates per row
NCORES = 8
QPC = N // NCORES             # 2048 queries per core
QTILES = QPC // 128           # 16
CH = 512                      # matmul slice = one PSUM bank of fp32
W = 1024                      # selection window = 2-bank PSUM tile
NWIN = N // W                 # 16
NCAND = NWIN * 8              # 128
GRP = 4                       # window tiles in flight (8 PSUM banks)
SC_Q = 512.0                  # query operand scale (2^9)
SC_D = 256.0                  # db operand scale (2^8)
ACT_SCALE = 2.0**-12          # psum*2^-12 -> 2^5 * dot
# X2 engine per window: True -> Pool (gpsimd, with ACT recast), else DVE.
# 9 of 16 windows on Pool, evenly spread.
POOL_WIN = [w % 16 in (0, 2, 4, 6, 8, 9, 11, 13, 15) for w in range(NWIN)]

_nc_cache = None


def _build():
    import concourse.bacc as bacc
    import concourse.mybir as mybir
    import concourse.tile as tile

    nc = bacc.Bacc(trn_type="TRN2")
    f32, f16 = mybir.dt.float32, mybir.dt.float16
    i32, u16 = mybir.dt.int32, mybir.dt.uint16
    Alu = mybir.AluOpType
    Act = mybir.ActivationFunctionType

    hq0_in = nc.dram_tensor("hq0", [128, QPC], f16, kind="ExternalInput")
    hq1_in = nc.dram_tensor("hq1", [128, QPC], f16, kind="ExternalInput")
    hT0_in = nc.dram_tensor("hT0", [128, N], f16, kind="ExternalInput")
    hT1_in = nc.dram_tensor("hT1", [128, N], f16, kind="ExternalInput")
    rowj_in = nc.dram_tensor("rowj", [128, N], f32, kind="ExternalInput")
    biasq_in = nc.dram_tensor("biasq", [128, QTILES], f32, kind="ExternalInput")
    out_i = nc.dram_tensor("out_i", [QPC, KDEV], i32, kind="ExternalOutput")

    with tile.TileContext(nc) as tc:
        with (
            tc.tile_pool(name="db", bufs=1) as db,
            tc.tile_pool(name="work", bufs=3) as work,
            tc.tile_pool(name="scp", bufs=3) as scp,
            tc.tile_pool(name="ppp", bufs=5) as ppp,
            tc.tile_pool(name="ps", bufs=GRP, space="PSUM") as ps,
        ):
            # ---------------- resident inputs ----------------
            hq = [db.tile([128, QPC], f16, name=f"hq{i}") for i in range(2)]
            nc.sync.dma_start(hq[0][:], hq0_in[:, :])
            nc.sync.dma_start(hq[1][:], hq1_in[:, :])
            hT = [db.tile([128, N], f16, name=f"hT{i}") for i in range(2)]
            rowj_sb = db.tile([128, N], f32, name="rowj")
            SL = 2048
            # interleave so window 0's dependencies arrive first
            for s0 in range(0, N, SL):
                sl = slice(s0, s0 + SL)
                nc.sync.dma_start(hT[0][:, sl], hT0_in[:, sl])
                nc.sync.dma_start(hT[1][:, sl], hT1_in[:, sl])
                nc.sync.dma_start(rowj_sb[:, sl], rowj_in[:, sl])
            biasq_sb = db.tile([128, QTILES], f32, name="biasq")
            nc.sync.dma_start(biasq_sb[:], biasq_in[:, :])

            # ---------------- constants ----------------
            c1023 = db.tile([128, 1], i32)
            nc.vector.memset(c1023[:], 1023)
            cfff8 = db.tile([128, 1], i32)
            nc.vector.memset(cfff8[:], 65528)      # 0xFFF8
            c128 = db.tile([128, 1], i32)
            nc.vector.memset(c128[:], W // 8)      # 128
            zero_i = db.tile([128, 1], i32)
            nc.vector.memset(zero_i[:], 0)
            zero_f = db.tile([128, 1], f32)
            nc.vector.memset(zero_f[:], 0.0)

            # ---------------- main loop over query tiles ----------------
            for t in range(QTILES):
                qs = slice(128 * t, 128 * (t + 1))
                v_cand = work.tile([128, NCAND], f32, tag="v_cand")
                import contextlib
                sc = (lambda nm: nc.named_scope(nm)) if t == 8 else (
                    lambda nm: contextlib.nullcontext())
                with sc("chunkstage"):
                 for g0 in range(0, NWIN, GRP):
                    wl = list(range(g0, min(NWIN, g0 + GRP)))
                    psums = [ps.tile([128, W], f32, tag="psum", name="psum")
                             for _ in wl]

                    def _ws(w, h):
                        return slice(W * w + CH * h, W * w + CH * (h + 1))
                    # pass-major: same stationary streams all slices
                    for i, w in enumerate(wl):
                        for h in range(2):
                            nc.tensor.matmul(
                                psums[i][:, CH * h:CH * (h + 1)],
                                hq[0][:, qs], hT[0][:, _ws(w, h)],
                                start=True, stop=False)
                    for i, w in enumerate(wl):
                        for h in range(2):
                            nc.tensor.matmul(
                                psums[i][:, CH * h:CH * (h + 1)],
                                hq[1][:, qs], hT[1][:, _ws(w, h)],
                                start=False, stop=True)
                    for i, w in enumerate(wl):
                        s_sb = scp.tile([128, W], i32, tag="s_sb",
                                        name="s_sb")
                        nc.scalar.activation(s_sb[:], psums[i][:],
                                             Act.Identity,
                                             bias=biasq_sb[:, t:t + 1],
                                             scale=ACT_SCALE)
                        p_sb = ppp.tile([128, W], f32, tag="p_sb",
                                        name="p_sb")
                        if POOL_WIN[w]:
                            s_f = ppp.tile([128, W], f32, tag="s_f",
                                           name="s_f")
                            nc.scalar.activation(s_f[:], s_sb[:],
                                                 Act.Identity,
                                                 bias=zero_f[:, 0:1],
                                                 scale=1.0)
                            for h in range(2):
                                hs = slice(CH * h, CH * (h + 1))
                                nc.gpsimd.tensor_add(p_sb[:, hs],
                                                     s_f[:, hs],
                                                     rowj_sb[:, _ws(w, h)])
                        else:
                            nc.vector.tensor_add(
                                p_sb[:], s_sb[:],
                                rowj_sb[:, W * w:W * (w + 1)])
                        nc.vector.max(out=v_cand[:, 8 * w:8 * w + 8],
                                      in_=p_sb[:])

                # merge: global top-40 of the candidate table (in place)
                with sc("merge"):
                    v40 = work.tile([128, KDEV], f32, tag="v40")
                    p_u = work.tile([128, KDEV], u16, tag="p_u")
                    for r in range(KDEV // 8):
                        nc.vector.max(out=v40[:, 8 * r:8 * r + 8],
                                      in_=v_cand[:])
                        nc.vector.max_index(
                            out=p_u[:, 8 * r:8 * r + 8],
                            in_max=v40[:, 8 * r:8 * r + 8],
                            in_values=v_cand[:],
                        )
                        if r < KDEV // 8 - 1:
                            nc.vector.match_replace(
                                out=v_cand[:],
                                in_to_replace=v40[:, 8 * r:8 * r + 8],
                                in_values=v_cand[:], imm_value=-3e38,
                            )

                # decode: global index = (p_u>>3)*1024 + (int(p*2048)&1023)
                with sc("decode"):
                    t32 = work.tile([128, KDEV], i32, tag="t32")
                    nc.scalar.activation(t32[:], v40[:], Act.Identity,
                                         bias=zero_f[:, 0:1], scale=2048.0)
                    j32 = work.tile([128, KDEV], i32, tag="j32")
                    nc.vector.scalar_tensor_tensor(
                        out=j32[:], in0=t32[:], scalar=c1023[:, 0:1],
                        in1=zero_i[:, 0:1].to_broadcast([128, KDEV]),
                        op0=Alu.bitwise_and, op1=Alu.bitwise_or,
                    )
                    pu32 = work.tile([128, KDEV], i32, tag="pu32")
                    nc.vector.tensor_copy(pu32[:], p_u[:])
                    m1 = work.tile([128, KDEV], i32, tag="m1")
                    nc.vector.scalar_tensor_tensor(
                        out=m1[:], in0=pu32[:], scalar=cfff8[:, 0:1],
                        in1=zero_i[:, 0:1].to_broadcast([128, KDEV]),
                        op0=Alu.bitwise_and, op1=Alu.bitwise_or,
                    )
                    gi = work.tile([128, KDEV], i32, tag="gi")
                    nc.vector.scalar_tensor_tensor(
                        out=gi[:], in0=m1[:], scalar=c128[:, 0:1],
                        in1=j32[:], op0=Alu.mult, op1=Alu.add,
                    )
                    # slot 0 is always the self-match: overwrite with row id
                    nc.gpsimd.iota(gi[:, 0:1], pattern=[[1, 1]], base=128 * t,
                                   channel_multiplier=1)

                nc.sync.dma_start(out_i[qs, :], gi[:])
    nc.finalize()
    return nc


def make_in_maps(x):
    """Host-side prep: fp16 operand splits + packed bias tables per core."""
    x = np.ascontiguousarray(np.asarray(x, dtype=np.float32))
    xT = x.T  # [256, N]
    h9 = (xT * np.float32(SC_Q)).astype(np.float16)   # query-side, scale 2^9
    h8 = (xT * np.float32(SC_D)).astype(np.float16)   # db-side, scale 2^8
    sq64 = (x.astype(np.float64) ** 2).sum(1)         # [N]

    # R_j = round(2^5*(128 - sq_j/2)) + j_local * 2^-11   (fp32-exact)
    rb = np.round((2.0**5) * (128.0 - sq64 / 2.0))
    rowj_row = (rb + (np.arange(N) % W) * (2.0**-11)).astype(np.float32)
    rowj = np.ascontiguousarray(np.broadcast_to(rowj_row, (128, N)))

    # biasq[i] = 2^5 * A_i,  A_i = (450 - sq_i)/2 - 128
    a_i = (450.0 - sq64) / 2.0 - 128.0
    biasq_full = ((2.0**5) * a_i).astype(np.float32)  # [N]

    in_maps = []
    for core in range(NCORES):
        qs = slice(core * QPC, (core + 1) * QPC)
        biasq = np.ascontiguousarray(
            biasq_full[qs].reshape(QTILES, 128).T)     # [128, QTILES]
        in_maps.append({
            "hq0": np.ascontiguousarray(h9[:128, qs]),
            "hq1": np.ascontiguousarray(h9[128:, qs]),
            "hT0": np.ascontiguousarray(h8[:128]),
            "hT1": np.ascontiguousarray(h8[128:]),
            "rowj": rowj,
            "biasq": biasq,
        })
    return in_maps


def postprocess(x, idx40):
    """Host refine: exact fp32 distances for 40 candidates, sort, keep 32."""
    idx40 = idx40.astype(np.int64)  # [N, KDEV]
    # slot 0 is always the self-match; the device writes core-local row ids,
    # so restore the global ids here.
    idx40[:, 0] = np.arange(N)
    np.clip(idx40, 0, N - 1, out=idx40)
    d40 = np.empty((N, KDEV), np.float32)
    for r0 in range(0, N, 1024):
        blk = slice(r0, min(N, r0 + 1024))
        diff = x[blk][:, None, :] - x[idx40[blk]]
        d40[blk] = (diff * diff).sum(-1)
    sidx = np.lexsort((idx40, d40), axis=1)[:, :K]
    idx = np.take_along_axis(idx40, sidx, axis=1).astype(np.int32)
    dist = np.take_along_axis(d40, sidx, axis=1).astype(np.float32)
    return idx, dist


def kernel(x, k):
    from concourse.bass_utils import run_bass_kernel_spmd

    global _nc_cache
    x = np.ascontiguousarray(np.asarray(x, dtype=np.float32))
    assert x.shape == (N, D)
    assert int(k) == K

    if _nc_cache is None:
        _nc_cache = _build()
    nc = _nc_cache

    in_maps = make_in_maps(x)
    res = run_bass_kernel_spmd(nc, in_maps, core_ids=list(range(NCORES)))
    idx40 = np.concatenate([r["out_i"] for r in res.results], axis=0)
    return postprocess(x, idx40)


# revision 16
# speedup vs baseline: 1.0517x; 1.0517x over previous
"""Exact self-kNN (k=32) on 8 TRN2 NeuronCores — 2-pass PE, cast-quantized
packed scores, 1024-wide selection windows.

Structure per 1024-column window (16 windows x 16 query tiles per core):
  PE    : psum = 2^17 * x_i . x_j   (four fp16 matmuls into a 2-bank
          [128,1024] PSUM tile; q*2^9, db*2^8)
  ScalarE: s = int32(psum * 2^-12 + 2^5*A_i)  (ONE 1024-wide activation:
          the int32 output cast IS the score quantizer; A_i =
          (450-sq_i)/2 - 128 is per-row centering, ranking-neutral)
  X2    : p = float(s) + R_j  (tensor_tensor add; R_j =
          round(2^5*(128 - sq_j/2)) + j_local*2^-11 packs the column bias
          in integer units and the 10-bit local index in the fraction).
          Per window either VectorE (one 1024-wide mixed i32+f32 TT) or
          Pool (ACT recast to f32 + two 512-wide TTs; Pool rejects mixed
          dtypes and >512-wide ops are pathological on it)
  VectorE: max8 over the 1024-wide window -> 8 packed candidates
So p = 2^5*((450 - d_ij)/2)_q + j_local*2^-11, |p| < 2^13, fp32-exact.
A 5-round max8/max_index/match_replace merge over the 128 candidates
gives top-40; global index = (pos>>3)*1024 + (int32(p*2048) & 1023).
Slot 0 is always the self match and is overwritten with the row id.

Why this shape: the PE throttles to ~1.2 GHz effective only when its duty
cycle exceeds ~50%; with just the two irreducible dot passes it stays
under that and streams at 2.4 GHz.  Measured 512-wide costs: ACT
psum->sbuf ~710 ns (1024-wide ~1137), DVE max8 ~593/512 (~1187/1024),
DVE TT ~602 (1224/1024), Pool TT ~1232-1639.  The window widening cuts
the DVE max8+merge floor; the 2-bank cast cuts ACT; the 9/16 pool share
keeps Pool below its concurrency ceiling.

Host: exact fp32 distances for the 40 candidates per query, stable
(d, idx) sort, keep 32.
"""

import numpy as np

N = 16384
D = 256
K = 32
KDEV = 40                     # device returns top-40 candidates per row
NCORES = 8
QPC = N // NCORES             # 2048 queries per core
QTILES = QPC // 128           # 16
CH = 512                      # matmul slice = one PSUM bank of fp32
W = 1024                      # selection window = 2-bank PSUM tile
NWIN = N // W                 # 16
NCAND = NWIN * 8              # 128
GRP = 4                       # window tiles in flight (8 PSUM banks)
SC_Q = 512.0                  # query operand scale (2^9)
SC_D = 256.0                  # db operand scale (2^8)
ACT_SCALE = 2.0**-12          # psum*2^-12 -> 2^5 * dot
# X2 engine per window: True -> Pool (gpsimd, with ACT recast), else DVE.
# 9 of 16 windows on Pool in pair-size bunches (pairs are scheduling-safe;
# long runs are not), with the qtile TAIL (w=13,14,15) on the fast DVE
# path so the merge's first op never waits on the slow Pool chain.
POOL_WIN = [w % 16 in (0, 1, 3, 4, 6, 7, 9, 10, 12) for w in range(NWIN)]

_nc_cache = None


def _build():
    import concourse.bacc as bacc
    import concourse.mybir as mybir
    import concourse.tile as tile

    nc = bacc.Bacc(trn_type="TRN2")
    f32, f16 = mybir.dt.float32, mybir.dt.float16
    i32, u16 = mybir.dt.int32, mybir.dt.uint16
    Alu = mybir.AluOpType
    Act = mybir.ActivationFunctionType

    hq0_in = nc.dram_tensor("hq0", [128, QPC], f16, kind="ExternalInput")
    hq1_in = nc.dram_tensor("hq1", [128, QPC], f16, kind="ExternalInput")
    hT0_in = nc.dram_tensor("hT0", [128, N], f16, kind="ExternalInput")
    hT1_in = nc.dram_tensor("hT1", [128, N], f16, kind="ExternalInput")
    rowj_in = nc.dram_tensor("rowj", [128, N], f32, kind="ExternalInput")
    biasq_in = nc.dram_tensor("biasq", [128, QTILES], f32, kind="ExternalInput")
    out_i = nc.dram_tensor("out_i", [QPC, KDEV], i32, kind="ExternalOutput")

    with tile.TileContext(nc) as tc:
        with (
            tc.tile_pool(name="db", bufs=1) as db,
            tc.tile_pool(name="work", bufs=3) as work,
            tc.tile_pool(name="scp", bufs=4) as scp,
            tc.tile_pool(name="ppp", bufs=4) as ppp,
            tc.tile_pool(name="ps", bufs=GRP, space="PSUM") as ps,
        ):
            # ---------------- resident inputs ----------------
            hq = [db.tile([128, QPC], f16, name=f"hq{i}") for i in range(2)]
            nc.sync.dma_start(hq[0][:], hq0_in[:, :])
            nc.sync.dma_start(hq[1][:], hq1_in[:, :])
            hT = [db.tile([128, N], f16, name=f"hT{i}") for i in range(2)]
            rowj_sb = db.tile([128, N], f32, name="rowj")
            SL = 2048
            # interleave so window 0's dependencies arrive first
            for s0 in range(0, N, SL):
                sl = slice(s0, s0 + SL)
                nc.sync.dma_start(hT[0][:, sl], hT0_in[:, sl])
                nc.sync.dma_start(hT[1][:, sl], hT1_in[:, sl])
                nc.sync.dma_start(rowj_sb[:, sl], rowj_in[:, sl])
            biasq_sb = db.tile([128, QTILES], f32, name="biasq")
            nc.sync.dma_start(biasq_sb[:], biasq_in[:, :])

            # ---------------- constants ----------------
            c1023 = db.tile([128, 1], i32)
            nc.vector.memset(c1023[:], 1023)
            cfff8 = db.tile([128, 1], i32)
            nc.vector.memset(cfff8[:], 65528)      # 0xFFF8
            c128 = db.tile([128, 1], i32)
            nc.vector.memset(c128[:], W // 8)      # 128
            zero_i = db.tile([128, 1], i32)
            nc.vector.memset(zero_i[:], 0)
            zero_f = db.tile([128, 1], f32)
            nc.vector.memset(zero_f[:], 0.0)

            # ---------------- main loop over query tiles ----------------
            for t in range(QTILES):
                qs = slice(128 * t, 128 * (t + 1))
                v_cand = work.tile([128, NCAND], f32, tag="v_cand")
                import contextlib
                sc = (lambda nm: nc.named_scope(nm)) if t == 8 else (
                    lambda nm: contextlib.nullcontext())
                with sc("chunkstage"):
                 for g0 in range(0, NWIN, GRP):
                    wl = list(range(g0, min(NWIN, g0 + GRP)))
                    psums = [ps.tile([128, W], f32, tag="psum", name="psum")
                             for _ in wl]

                    def _ws(w, h):
                        return slice(W * w + CH * h, W * w + CH * (h + 1))
                    # pass-major: same stationary streams all slices
                    for i, w in enumerate(wl):
                        for h in range(2):
                            nc.tensor.matmul(
                                psums[i][:, CH * h:CH * (h + 1)],
                                hq[0][:, qs], hT[0][:, _ws(w, h)],
                                start=True, stop=False)
                    for i, w in enumerate(wl):
                        for h in range(2):
                            nc.tensor.matmul(
                                psums[i][:, CH * h:CH * (h + 1)],
                                hq[1][:, qs], hT[1][:, _ws(w, h)],
                                start=False, stop=True)
                    for i, w in enumerate(wl):
                        s_sb = scp.tile([128, W], i32, tag="s_sb",
                                        name="s_sb")
                        nc.scalar.activation(s_sb[:], psums[i][:],
                                             Act.Identity,
                                             bias=biasq_sb[:, t:t + 1],
                                             scale=ACT_SCALE)
                        p_sb = ppp.tile([128, W], f32, tag="p_sb",
                                        name="p_sb")
                        if POOL_WIN[w]:
                            s_f = ppp.tile([128, W], f32, tag="s_f",
                                           name="s_f")
                            nc.scalar.activation(s_f[:], s_sb[:],
                                                 Act.Identity,
                                                 bias=zero_f[:, 0:1],
                                                 scale=1.0)
                            for h in range(2):
                                hs = slice(CH * h, CH * (h + 1))
                                nc.gpsimd.tensor_add(p_sb[:, hs],
                                                     s_f[:, hs],
                                                     rowj_sb[:, _ws(w, h)])
                        else:
                            nc.vector.tensor_add(
                                p_sb[:], s_sb[:],
                                rowj_sb[:, W * w:W * (w + 1)])
                        nc.vector.max(out=v_cand[:, 8 * w:8 * w + 8],
                                      in_=p_sb[:])

                # merge: global top-40 of the candidate table (in place)
                with sc("merge"):
                    v40 = work.tile([128, KDEV], f32, tag="v40")
                    p_u = work.tile([128, KDEV], u16, tag="p_u")
                    for r in range(KDEV // 8):
                        nc.vector.max(out=v40[:, 8 * r:8 * r + 8],
                                      in_=v_cand[:])
                        nc.vector.max_index(
                            out=p_u[:, 8 * r:8 * r + 8],
                            in_max=v40[:, 8 * r:8 * r + 8],
                            in_values=v_cand[:],
                        )
                        if r < KDEV // 8 - 1:
                            nc.vector.match_replace(
                                out=v_cand[:],
                                in_to_replace=v40[:, 8 * r:8 * r + 8],
                                in_values=v_cand[:], imm_value=-3e38,
                            )

                # decode: global index = (p_u>>3)*1024 + (int(p*2048)&1023)
                with sc("decode"):
                    t32 = work.tile([128, KDEV], i32, tag="t32")
                    nc.vector.tensor_scalar_mul(t32[:], v40[:], 2048.0)
                    j32 = work.tile([128, KDEV], i32, tag="j32")
                    nc.vector.scalar_tensor_tensor(
                        out=j32[:], in0=t32[:], scalar=c1023[:, 0:1],
                        in1=zero_i[:, 0:1].to_broadcast([128, KDEV]),
                        op0=Alu.bitwise_and, op1=Alu.bitwise_or,
                    )
                    pu32 = work.tile([128, KDEV], i32, tag="pu32")
                    nc.vector.tensor_copy(pu32[:], p_u[:])
                    m1 = work.tile([128, KDEV], i32, tag="m1")
                    nc.vector.scalar_tensor_tensor(
                        out=m1[:], in0=pu32[:], scalar=cfff8[:, 0:1],
                        in1=zero_i[:, 0:1].to_broadcast([128, KDEV]),
                        op0=Alu.bitwise_and, op1=Alu.bitwise_or,
                    )
                    gi = work.tile([128, KDEV], i32, tag="gi")
                    nc.vector.scalar_tensor_tensor(
                        out=gi[:], in0=m1[:], scalar=c128[:, 0:1],
                        in1=j32[:], op0=Alu.mult, op1=Alu.add,
                    )
                    # slot 0 is always the self-match: overwrite with row id
                    nc.gpsimd.iota(gi[:, 0:1], pattern=[[1, 1]], base=128 * t,
                                   channel_multiplier=1)

                nc.sync.dma_start(out_i[qs, :], gi[:])
    nc.finalize()
    return nc


def make_in_maps(x):
    """Host-side prep: fp16 operand splits + packed bias tables per core."""
    x = np.ascontiguousarray(np.asarray(x, dtype=np.float32))
    xT = x.T  # [256, N]
    h9 = (xT * np.float32(SC_Q)).astype(np.float16)   # query-side, scale 2^9
    h8 = (xT * np.float32(SC_D)).astype(np.float16)   # db-side, scale 2^8
    sq64 = (x.astype(np.float64) ** 2).sum(1)         # [N]

    # R_j = round(2^5*(128 - sq_j/2)) + j_local * 2^-11   (fp32-exact)
    rb = np.round((2.0**5) * (128.0 - sq64 / 2.0))
    rowj_row = (rb + (np.arange(N) % W) * (2.0**-11)).astype(np.float32)
    rowj = np.ascontiguousarray(np.broadcast_to(rowj_row, (128, N)))

    # biasq[i] = 2^5 * A_i,  A_i = (450 - sq_i)/2 - 128
    a_i = (450.0 - sq64) / 2.0 - 128.0
    biasq_full = ((2.0**5) * a_i).astype(np.float32)  # [N]

    in_maps = []
    for core in range(NCORES):
        qs = slice(core * QPC, (core + 1) * QPC)
        biasq = np.ascontiguousarray(
            biasq_full[qs].reshape(QTILES, 128).T)     # [128, QTILES]
        in_maps.append({
            "hq0": np.ascontiguousarray(h9[:128, qs]),
            "hq1": np.ascontiguousarray(h9[128:, qs]),
            "hT0": np.ascontiguousarray(h8[:128]),
            "hT1": np.ascontiguousarray(h8[128:]),
            "rowj": rowj,
            "biasq": biasq,
        })
    return in_maps


def postprocess(x, idx40):
    """Host refine: exact fp32 distances for 40 candidates, sort, keep 32."""
    idx40 = idx40.astype(np.int64)  # [N, KDEV]
    # slot 0 is always the self-match; the device writes core-local row ids,
    # so restore the global ids here.
    idx40[:, 0] = np.arange(N)
    np.clip(idx40, 0, N - 1, out=idx40)
    d40 = np.empty((N, KDEV), np.float32)
    for r0 in range(0, N, 1024):
        blk = slice(r0, min(N, r0 + 1024))
        diff = x[blk][:, None, :] - x[idx40[blk]]
        d40[blk] = (diff * diff).sum(-1)
    sidx = np.lexsort((idx40, d40), axis=1)[:, :K]
    idx = np.take_along_axis(idx40, sidx, axis=1).astype(np.int32)
    dist = np.take_along_axis(d40, sidx, axis=1).astype(np.float32)
    return idx, dist


def kernel(x, k):
    from concourse.bass_utils import run_bass_kernel_spmd

    global _nc_cache
    x = np.ascontiguousarray(np.asarray(x, dtype=np.float32))
    assert x.shape == (N, D)
    assert int(k) == K

    if _nc_cache is None:
        _nc_cache = _build()
    nc = _nc_cache

    in_maps = make_in_maps(x)
    res = run_bass_kernel_spmd(nc, in_maps, core_ids=list(range(NCORES)))
    idx40 = np.concatenate([r["out_i"] for r in res.results], axis=0)
    return postprocess(x, idx40)
